# revision 14
# baseline (speedup 1.0000x reference)
"""FAPE loss kernel for Trainium2 (Bass/Tile), 8 NeuronCores.

Problem: B=8, N=1024.  reference computes, per batch b:
    R_i, t_i = backbone frames from (n, ca, c)          [N,3,3],[N,3]
    diff[i,j] = || R_i^T (pred_j - t_i) - R_i^T (true_j - t_i) ||
    per_pair  = min(diff,10) + 0.5*(diff - min(diff,10)) = 0.5*(diff + min(diff,10))
    out = sum_b sum_ij m_i m_j per_pair / (sum(m) + 1e-8)

Key identity: both pred and true are expressed in the SAME frame i, so
    R_i^T (pred_j - t_i) - R_i^T (true_j - t_i) = R_i^T d_j,  d_j = pred_j - true_j
and R_i is orthonormal by construction (x, y, z mutually orthogonal unit
vectors from normalized cross products), hence
    diff[i,j] = ||R_i^T d_j|| = ||d_j||            (independent of i!)
up to the 1e-8 normalize-eps and f32 rounding (~1e-7 relative, verified
6e-7 end-to-end vs the jax reference; tolerance is 2e-2).  The O(N^2)
pairwise reduction therefore factorizes exactly:
    sum_ij m_i m_j f(||d_j||) = (sum_i m_i) * (sum_j m_j f(||d_j||))
leaving O(N) device work per batch: one masked norm + clamp + row-sum.

Per-core body (one batch per core, j = 8*p + t, 9 instructions).  The
host only needs the single scalar sum_j m_j*(diff_j + min(diff_j,10)),
so clamp+add fuse into one scalar_tensor_tensor and the whole thing
funnels through one PE partition-reduce:
    d    = pred - true                       [128,8,3]  DVE
    sq   = d*d                               [128,8,3]  Pool
    nsq  = reduce_X(sq)                      [128,8]    DVE
    nsqm = nsq * mask   (m in {0,1}: sqrt(m*nsq) = m*diff)       DVE
    dm   = sqrt(nsqm)           (= m*diff)   [128,8]    ACT
    s    = (dm min 10) + dm                  [128,8]    DVE
    ps   = ones[128,1]^T @ s[128,8] -> [1,8]  (partition sum)    PE
    sbo  = reduce_X(ps) -> [1,1]                                 DVE
    DMA sbo [1,1] -> HBM  (single 4-byte descriptor, SP-issued HWDGE;
    Pool-issued DMA is SWDGE whose ~1us descriptor generation runs on
    the Pool engine itself)
Host scales by 0.5 and the (sum_i m_i) factor per batch (f64), and
normalizes by sum(m) + 1e-8.

The bench loop (reps>0) unrolls UNROLL bodies per For_i iteration: the
For_i back-edge runs an all-engine barrier (~1.3us) that would otherwise
dwarf the body; unrolling amortizes it and lets consecutive bodies
pipeline through the tile-pool buffer rotation, so the measured slope is
the true steady-state per-body cost.

Sharding: batch-parallel, one batch per core (spec hint allows B data-parallel).
"""

import numpy as np

P = 128          # partitions
T = 8            # j = 8*p + t  (p-major; any index bijection works for the sum)
N = 1024
B = 8
NCORES = 8
UNROLL = 40

_cache: dict = {}


def _build_nc(reps=0, prep_only=False):
    """Emit the single-core BIR module (same NEFF runs SPMD on all 8 cores)."""
    from contextlib import ExitStack

    import concourse.bacc as bacc
    import concourse.mybir as mybir
    import concourse.tile as tile
    from concourse._compat import axon_active

    f32 = mybir.dt.float32
    Alu = mybir.AluOpType
    Act = mybir.ActivationFunctionType
    AxX = mybir.AxisListType.X

    nc = bacc.Bacc(
        "TRN2",
        target_bir_lowering=False,
        debug=not axon_active(),
        num_devices=NCORES,
    )

    # One concatenated input: cols [pred(3) true(3) mask(1) pad(1)]
    d_all = nc.dram_tensor("all_in", [N, 8], f32, kind="ExternalInput")
    # UNROLL rows so unrolled bench bodies write distinct addresses (no
    # artificial WAW chain between their DMAs); the real kernel and the
    # host only use row 0.
    d_out = nc.dram_tensor("out_acc", [UNROLL, 1], f32, kind="ExternalOutput")

    with tile.TileContext(nc) as tc, ExitStack() as ctx:
        sb = ctx.enter_context(tc.tile_pool(name="sb", bufs=1))
        wpool = ctx.enter_context(tc.tile_pool(name="wpool", bufs=6))
        opool = ctx.enter_context(tc.tile_pool(name="opool", bufs=6))
        pspool = ctx.enter_context(tc.tile_pool(name="pspool", bufs=8,
                                                space="PSUM"))

        # ---- ACT table warmup: force the sqrt set load early (overlaps DMA)
        warm = sb.tile([1, 2], f32)
        nc.vector.memset(warm[:], 1.0)
        nc.scalar.activation(warm[:, 1:2], warm[:, 0:1], Act.Sqrt)

        ones = sb.tile([P, 1], f32)
        nc.vector.memset(ones[:], 1.0)

        # ---- ONE input DMA: [1024,8] -> [128, 8, 8], j = 8*p + t.
        # Fully contiguous in DRAM, 256B per partition.
        stg = sb.tile([P, T, 8], f32)
        nc.sync.dma_start(stg[:], d_all.ap().rearrange("(p t) c -> p t c", p=P))

        t_pred = stg[:, :, 0:3]
        t_true = stg[:, :, 3:6]
        t_mask = stg[:, :, 6]

        def body(row=0):
            d = wpool.tile([P, T, 3], f32, tag="d", name="d")
            nc.vector.tensor_tensor(d[:], t_pred, t_true, Alu.subtract)
            sq = wpool.tile([P, T, 3], f32, tag="sq", name="sq")
            nc.gpsimd.tensor_tensor(sq[:], d[:], d[:], Alu.mult)
            nsq = wpool.tile([P, T], f32, tag="nsq", name="nsq")
            nc.vector.tensor_reduce(nsq[:], sq[:], AxX, Alu.add)
            nsqm = wpool.tile([P, T], f32, tag="nsqm", name="nsqm")
            nc.vector.tensor_tensor(nsqm[:], nsq[:], t_mask, Alu.mult)

            dm = wpool.tile([P, T], f32, tag="dm", name="dm")
            nc.scalar.activation(dm[:], nsqm[:], Act.Sqrt)
            s = wpool.tile([P, T], f32, tag="s", name="s")
            nc.vector.scalar_tensor_tensor(s[:], dm[:], 10.0, dm[:],
                                           Alu.min, Alu.add)

            ps = pspool.tile([1, T], f32, tag="ps", name="ps")
            nc.tensor.matmul(ps[:], ones[:], s[:], start=True, stop=True)
            sbo = opool.tile([1, 1], f32, tag="sbo", name="sbo")
            nc.vector.tensor_reduce(sbo[:], ps[:], AxX, Alu.add)
            nc.sync.dma_start(d_out.ap()[row:row + 1, :], sbo[:])

        if reps:
            assert reps % UNROLL == 0, f"reps must be a multiple of {UNROLL}"
            with tc.For_i(0, reps // UNROLL, 1):
                for u in range(UNROLL):
                    body(u)
        else:
            body()

    nc.compile()
    return nc


def _get_nc():
    if "nc" not in _cache:
        _cache["nc"] = _build_nc()
    return _cache["nc"]


def make_inmaps(n, ca, c, pred_pos, true_pos, mask):
    allc = np.empty((B, N, 8), np.float32)
    allc[:, :, 0:3] = np.asarray(pred_pos, np.float32)
    allc[:, :, 3:6] = np.asarray(true_pos, np.float32)
    allc[:, :, 6] = np.asarray(mask).astype(np.float32)
    allc[:, :, 7] = 0.0
    return [{"all_in": allc[b]} for b in range(B)]


def kernel(n, ca, c, pred_pos, true_pos, mask) -> np.ndarray:
    from concourse.bass_utils import run_bass_kernel_spmd

    nc = _get_nc()
    in_maps = make_inmaps(n, ca, c, pred_pos, true_pos, mask)
    res = run_bass_kernel_spmd(nc, in_maps, core_ids=list(range(NCORES)))
    m = np.asarray(mask).astype(np.float64)
    c_b = m.sum(axis=1)                      # per-batch masked-residue count
    total = 0.0
    for b in range(B):
        sheet = res.results[b]["out_acc"][0].astype(np.float64)
        total += c_b[b] * 0.5 * sheet.sum()
    return np.asarray(total / (m.sum() + 1e-8), dtype=np.float32)
